# revision 2
# baseline (speedup 1.0000x reference)
"""Trainium2 Bass kernel for nn_AE_RNN: 2-layer GRU AE-RNN, scalar SSE loss.

Data-parallel over batch (256 -> 32 per core, 8 cores). Host sums partials.

Device program per core (SPMD):
  - bulk:  relu1 = relu(W1.u + b1)                      [big matmuls over C*B]
           G0 = Wg0.relu1 + bg0   (composed wih0 @ pu_w2; z-gate negated)
  - steps: 2-layer GRU recurrence (the only sequential part)
  - tail:  dphi -> xcat -> phi_x -> me -> yhat -> loss  (composed weights)
All bulk/step matmul operands bf16, psum f32, activations f32-internal.
"""

import os
import numpy as np
import ml_dtypes

import concourse.bass as bass
import concourse.bacc as bacc
import concourse.mybir as mybir
import concourse.tile as tile
from concourse.bass_utils import run_bass_kernel_spmd

BF16 = mybir.dt.bfloat16
F32 = mybir.dt.float32
AF = mybir.ActivationFunctionType
OP = mybir.AluOpType

B, T = 256, 2048
U = Y = 32
H = 128
Z = 64
NCORE = 8
BL = B // NCORE  # 32 batch rows per core

C = 128  # steps per chunk
NCH = T // C


def _bf(x):
    return np.asarray(x, dtype=ml_dtypes.bfloat16)


def build_nc(nch=NCH):
    nc = bacc.Bacc("TRN2", target_bir_lowering=False, debug=False)
    Tl = nch * C
    NT = (C * BL) // 512  # 512-wide tiles per chunk = 8

    u_d = nc.dram_tensor("u", [U, Tl, BL], BF16, kind="ExternalInput")
    y_d = nc.dram_tensor("y", [Y, Tl, BL], BF16, kind="ExternalInput")
    h00_d = nc.dram_tensor("h00", [H, BL], F32, kind="ExternalInput")
    h01_d = nc.dram_tensor("h01", [H, BL], F32, kind="ExternalInput")
    wp_d = nc.dram_tensor("wpack", [128, 2464], BF16, kind="ExternalInput")
    bp_d = nc.dram_tensor("bpack", [128, 16], F32, kind="ExternalInput")
    out_d = nc.dram_tensor("out", [1, 1], F32, kind="ExternalOutput")

    with tile.TileContext(nc) as tc:
        with (
            tc.tile_pool(name="wt", bufs=1) as wtp,
            tc.tile_pool(name="io", bufs=3) as iop,
            tc.tile_pool(name="big", bufs=2) as bigp,
            tc.tile_pool(name="sm", bufs=4) as smp,
            tc.tile_pool(name="st", bufs=12) as stp,
            tc.tile_pool(name="ps0", bufs=2, space="PSUM") as pss0,
            tc.tile_pool(name="ps1", bufs=2, space="PSUM") as pss1,
            tc.tile_pool(name="psb", bufs=2, space="PSUM") as psb,
        ):
            wp = wtp.tile([128, 2464], BF16)
            bp = wtp.tile([128, 16], F32)
            nc.sync.dma_start(wp[:], wp_d[:])
            nc.sync.dma_start(bp[:], bp_d[:])

            def W(i, n=128):
                return wp[:, i * 128:i * 128 + n]

            def Bc(j, p=128):
                return bp[:p, j:j + 1]

            ones32 = wtp.tile([32, 1], F32)
            nc.vector.memset(ones32[:], 1.0)
            onesb = wtp.tile([128, BL], BF16)
            nc.vector.memset(onesb[:], 1.0)

            h0f = wtp.tile([H, BL], F32)
            h1f = wtp.tile([H, BL], F32)
            nc.sync.dma_start(h0f[:], h00_d[:])
            nc.sync.dma_start(h1f[:], h01_d[:])
            h_init0 = wtp.tile([H, BL], BF16)
            h_init1 = wtp.tile([H, BL], BF16)
            nc.vector.tensor_copy(h_init0[:], h0f[:])
            nc.vector.tensor_copy(h_init1[:], h1f[:])

            loss_acc = wtp.tile([32, NT * nch], F32)
            nc.vector.memset(loss_acc[:], 0.0)

            h0_prev = h_init0[:]
            h1carry_prev = h_init1
            prev = None  # (relu1, H1S, Yt) of previous chunk

            for k in range(nch):
                Ut = iop.tile([U, C * BL], BF16, tag="ut")
                Yt = iop.tile([Y, C * BL], BF16, tag="yt")
                nc.sync.dma_start(
                    Ut[:].rearrange("p (t b) -> p t b", b=BL),
                    u_d[:, k * C:(k + 1) * C, :])
                nc.sync.dma_start(
                    Yt[:].rearrange("p (t b) -> p t b", b=BL),
                    y_d[:, k * C:(k + 1) * C, :])

                # ---- bulk: relu1 = relu(W1 u + b1) ----
                relu1 = bigp.tile([H, C * BL], BF16, tag="relu1")
                for j in range(NT):
                    ps = psb.tile([128, 512], F32, tag="pb")
                    nc.tensor.matmul(ps[:], W(0)[0:32, :],
                                     Ut[:, j * 512:(j + 1) * 512])
                    nc.scalar.activation(relu1[:, j * 512:(j + 1) * 512], ps[:],
                                         AF.Relu, bias=Bc(0))

                # ---- bulk: G0 gates; r/zneg interleaved as per-step [r|z] ----
                G0rz = bigp.tile([H, C * 64], BF16, tag="g0rz")
                G0n = bigp.tile([H, C * BL], BF16, tag="g0n")
                G0rz3 = G0rz[:].rearrange("p (t b) -> p t b", b=64)
                for g in range(3):
                    for j in range(NT):
                        ps = psb.tile([128, 512], F32, tag="pb")
                        nc.tensor.matmul(ps[:], W(1 + g),
                                         relu1[:, j * 512:(j + 1) * 512])
                        if g < 2:
                            dst = G0rz3[:, j * 16:(j + 1) * 16,
                                        g * 32:(g + 1) * 32]
                            nc.scalar.activation(
                                dst, ps[:].rearrange("p (t b) -> p t b", b=32),
                                AF.Identity, bias=Bc(1 + g))
                        else:
                            nc.vector.tensor_scalar(
                                G0n[:, j * 512:(j + 1) * 512], ps[:],
                                Bc(3), None, OP.add)

                # ---- H1S shifted hidden buffer (slot t = h1_{t-1}) ----
                H1S = bigp.tile([H, (C + 1) * BL], BF16, tag="h1s")
                nc.vector.tensor_copy(H1S[:, 0:BL], h1carry_prev[:])

                # ---- GRU steps ----
                _steps = range(0 if os.environ.get("KABL") in ("bulk",) else C)
                for t in _steps:
                    h1_prev = H1S[:, t * BL:(t + 1) * BL]
                    # ===== layer 0 =====
                    psA = pss0.tile([128, 128], F32, tag="psA")
                    nc.tensor.matmul(psA[:, 0:32], W(4), h0_prev)
                    nc.tensor.matmul(psA[:, 32:64], W(5), h0_prev)
                    nc.vector.tensor_tensor(
                        psA[:, 0:64], psA[:, 0:64],
                        G0rz[:, t * 64:(t + 1) * 64], op=OP.add)
                    rz0 = stp.tile([128, 64], BF16, tag="rz0s")
                    nc.scalar.activation(rz0[:], psA[:, 0:64], AF.Sigmoid)
                    nc.tensor.matmul(psA[:, 64:96], W(6), h0_prev)
                    tn = stp.tile([128, 32], BF16, tag="tn0")
                    nc.vector.tensor_tensor(tn[:], rz0[:, 0:32], psA[:, 64:96],
                                            op=OP.mult)
                    nc.vector.tensor_tensor(psA[:, 96:128], tn[:],
                                            G0n[:, t * 32:(t + 1) * 32],
                                            op=OP.add)
                    n0 = stp.tile([128, 32], BF16, tag="nn0")
                    nc.scalar.activation(n0[:], psA[:, 96:128], AF.Tanh)
                    # blend: h = zc*n + (1-zc)*h_prev ; rz0[:,32:64] is zc
                    z0 = stp.tile([128, 32], BF16, tag="z0")
                    nc.gpsimd.tensor_tensor(z0[:], onesb[:, 0:32],
                                            rz0[:, 32:64], op=OP.subtract)
                    b0 = stp.tile([128, 32], BF16, tag="b0")
                    nc.gpsimd.tensor_tensor(b0[:], z0[:], h0_prev, op=OP.mult)
                    a0 = stp.tile([128, 32], BF16, tag="a0")
                    nc.vector.tensor_tensor(a0[:], rz0[:, 32:64], n0[:],
                                            op=OP.mult)
                    h0_new = stp.tile([128, 32], BF16, tag="h0")
                    nc.vector.tensor_tensor(h0_new[:], a0[:], b0[:], op=OP.add)

                    # ===== layer 1 ===== (gi via in-step matmuls)
                    psB = pss1.tile([128, 128], F32, tag="psB")
                    nc.tensor.matmul(psB[:, 0:32], W(10), h1_prev,
                                     start=True, stop=False)
                    nc.tensor.matmul(psB[:, 0:32], W(7), h0_new[:],
                                     start=False, stop=True)
                    nc.tensor.matmul(psB[:, 32:64], W(11), h1_prev,
                                     start=True, stop=False)
                    nc.tensor.matmul(psB[:, 32:64], W(8), h0_new[:],
                                     start=False, stop=True)
                    rz1 = stp.tile([128, 64], BF16, tag="rz1s")
                    nc.scalar.activation(rz1[:], psB[:, 0:64], AF.Sigmoid)
                    nc.tensor.matmul(psB[:, 64:96], W(12), h1_prev)
                    nc.tensor.matmul(psB[:, 96:128], W(9), h0_new[:])
                    tn1 = stp.tile([128, 32], BF16, tag="tn1")
                    nc.vector.tensor_tensor(tn1[:], rz1[:, 0:32], psB[:, 64:96],
                                            op=OP.mult)
                    nc.vector.tensor_tensor(psB[:, 96:128], tn1[:],
                                            psB[:, 96:128], op=OP.add)
                    n1 = stp.tile([128, 32], BF16, tag="nn1")
                    nc.scalar.activation(n1[:], psB[:, 96:128], AF.Tanh)
                    z1 = stp.tile([128, 32], BF16, tag="z1")
                    nc.gpsimd.tensor_tensor(z1[:], onesb[:, 0:32],
                                            rz1[:, 32:64], op=OP.subtract)
                    b1t = stp.tile([128, 32], BF16, tag="b1")
                    nc.gpsimd.tensor_tensor(b1t[:], z1[:], h1_prev, op=OP.mult)
                    a1 = stp.tile([128, 32], BF16, tag="a1")
                    nc.vector.tensor_tensor(a1[:], rz1[:, 32:64], n1[:],
                                            op=OP.mult)
                    nc.vector.tensor_tensor(
                        H1S[:, (t + 1) * BL:(t + 2) * BL], a1[:], b1t[:],
                        op=OP.add)
                    h0_prev = h0_new[:]

                h1carry = smp.tile([H, BL], BF16, tag="h1c")
                nc.vector.tensor_copy(h1carry[:], H1S[:, C * BL:(C + 1) * BL])
                h1carry_prev = h1carry

                if os.environ.get("KABL") in ("bulk", "steps"):
                    prev = None
                    # produce a dependency on bulk/step outputs
                    dsum = smp.tile([128, 1], F32, tag="dsum")
                    nc.vector.tensor_reduce(dsum[:], G0n[:, 0:512] if os.environ.get("KABL") == "bulk" else H1S[:, 0:512],
                                            axis=mybir.AxisListType.X, op=OP.add)
                    nc.vector.tensor_copy(loss_acc[:, k:k+1], dsum[0:32, :])
                    continue
                if prev is not None:
                    _tail(nc, prev, W, Bc, smp, psb, loss_acc, k - 1, NT)
                prev = (relu1, H1S, Yt)

            if prev is not None:
                _tail(nc, prev, W, Bc, smp, psb, loss_acc, nch - 1, NT)

            # ---- final loss reduction ----
            lsum = smp.tile([32, 1], F32, tag="lsum")
            nc.vector.tensor_reduce(lsum[:], loss_acc[:],
                                    axis=mybir.AxisListType.X, op=OP.add)
            psl = psb.tile([1, 1], F32, tag="pb5")
            nc.tensor.matmul(psl[:], ones32[:], lsum[:])
            lout = smp.tile([1, 1], F32, tag="lout")
            nc.vector.tensor_copy(lout[:], psl[:])
            nc.sync.dma_start(out_d[:], lout[:])

    nc.compile()
    return nc


def _tail(nc, prev, W, Bc, smp, psb, loss_acc, k, NT):
    relu1, H1S, Yt = prev
    for j in range(NT):
        sl = slice(j * 512, (j + 1) * 512)
        ps = psb.tile([128, 512], F32, tag="pb")
        nc.tensor.matmul(ps[:], W(13), relu1[:, sl], start=True, stop=False)
        nc.tensor.matmul(ps[:], W(14), H1S[:, sl], start=False, stop=True)
        dphi = smp.tile([128, 512], BF16, tag="dphi")
        nc.scalar.activation(dphi[:], ps[:], AF.Relu, bias=Bc(4))
        ps2 = psb.tile([128, 512], F32, tag="pb")
        nc.tensor.matmul(ps2[:], W(15), dphi[:])
        xcat = smp.tile([128, 512], BF16, tag="xcat")
        nc.scalar.activation(xcat[:], ps2[:], AF.Identity, bias=Bc(5))
        ps3 = psb.tile([128, 512], F32, tag="pb")
        nc.tensor.matmul(ps3[:], W(16), xcat[:])
        px = smp.tile([128, 512], BF16, tag="px")
        nc.scalar.activation(px[:], ps3[:], AF.Relu, bias=Bc(6))
        ps4 = psb.tile([128, 512], F32, tag="pb")
        nc.tensor.matmul(ps4[:], W(17), px[:])
        me = smp.tile([128, 512], BF16, tag="me")
        nc.scalar.activation(me[:], ps4[:], AF.Relu, bias=Bc(7))
        ps5 = psb.tile([32, 512], F32, tag="pb5")
        nc.tensor.matmul(ps5[:], W(18, 32), me[:])
        d = smp.tile([32, 512], F32, tag="dd")
        nc.vector.scalar_tensor_tensor(d[:], ps5[:], Bc(8, 32), Yt[:, sl],
                                       op0=OP.add, op1=OP.subtract)
        d2 = smp.tile([32, 512], F32, tag="d2")
        nc.scalar.activation(d2[:], d[:], AF.Square,
                             accum_out=loss_acc[:, k * NT + j:k * NT + j + 1])


def prep_inputs(u, y, h0, pu_w1, pu_b1, pu_w2, pu_b2, dy_w1, dy_b1, dy_w2,
                dy_b2, xm_w, xm_b, xlv_w, xlv_b, px_w1, px_b1, px_w2, px_b2,
                me_w1, me_b1, me_w2, me_b2, gru_wih, gru_whh, t_steps=T):
    """Host-side: compose weights, build wpack/bpack, per-core input shards."""
    f = np.float32
    asf = lambda x: np.array(x, f)
    (u, y, h0, pu_w1, pu_b1, pu_w2, pu_b2, dy_w1, dy_b1, dy_w2, dy_b2, xm_w,
     xm_b, xlv_w, xlv_b, px_w1, px_b1, px_w2, px_b2, me_w1, me_b1, me_w2,
     me_b2, gru_wih, gru_whh) = map(asf, (
         u, y, h0, pu_w1, pu_b1, pu_w2, pu_b2, dy_w1, dy_b1, dy_w2, dy_b2,
         xm_w, xm_b, xlv_w, xlv_b, px_w1, px_b1, px_w2, px_b2, me_w1, me_b1,
         me_w2, me_b2, gru_wih, gru_whh))
    wih0, wih1 = gru_wih[0].copy(), gru_wih[1].copy()
    whh0, whh1 = gru_whh[0].copy(), gru_whh[1].copy()
    # negate z-gate (rows H:2H) so sigmoid gives zc = 1-z
    for w in (wih0, whh0, wih1, whh1):
        w[H:2 * H] *= -1.0

    Wg0 = wih0 @ pu_w2
    bg0 = wih0 @ pu_b2
    Wda = dy_w1[:, :H] @ pu_w2
    bda = dy_w1[:, :H] @ pu_b2 + dy_b1
    dyw1b = dy_w1[:, H:]
    Wxc = np.vstack([xm_w, xlv_w])
    WxcP = Wxc @ dy_w2
    bxcP = Wxc @ dy_b2 + np.concatenate([xm_b, xlv_b])
    WmeP = me_w1 @ px_w2
    bmeP = me_w1 @ px_b2 + me_b1

    wpack = np.zeros((128, 2464), f)

    def put(i, w):  # w: [out, in] -> lhsT [in, out]
        wt = np.ascontiguousarray(w.T)
        wpack[:wt.shape[0], i * 128:i * 128 + wt.shape[1]] = wt

    put(0, pu_w1)
    for g in range(3):
        put(1 + g, Wg0[g * H:(g + 1) * H])
        put(4 + g, whh0[g * H:(g + 1) * H])
        put(7 + g, wih1[g * H:(g + 1) * H])
        put(10 + g, whh1[g * H:(g + 1) * H])
    put(13, Wda)
    put(14, dyw1b)
    put(15, WxcP)
    put(16, px_w1)
    put(17, WmeP)
    put(18, me_w2)

    bpack = np.zeros((128, 16), f)
    for j, b in enumerate([pu_b1, bg0[:H], bg0[H:2 * H], bg0[2 * H:],
                           bda, bxcP, px_b1, bmeP, me_b2]):
        bpack[:len(b), j] = b

    in_maps = []
    for c in range(NCORE):
        rows = slice(c * BL, (c + 1) * BL)
        in_maps.append({
            "u": _bf(u[rows, :, :t_steps].transpose(1, 2, 0)),  # [U, T, BL]
            "y": _bf(y[rows, :, :t_steps].transpose(1, 2, 0)),
            "h00": np.ascontiguousarray(h0[0, rows].T),
            "h01": np.ascontiguousarray(h0[1, rows].T),
            "wpack": _bf(wpack),
            "bpack": bpack,
        })
    return in_maps


_NC_CACHE = {}


def run_on_cores(in_maps, nch=NCH, trace=False, tmpdir=None):
    key = nch
    if key not in _NC_CACHE:
        _NC_CACHE[key] = build_nc(nch)
    nc = _NC_CACHE[key]
    res = run_bass_kernel_spmd(nc, in_maps, core_ids=list(range(NCORE)),
                               trace=trace, tmpdir=tmpdir)
    total = np.float32(0.0)
    for r in res.results:
        total += np.float32(r["out"][0, 0])
    return np.float32(total), res


def kernel(**inputs):
    in_maps = prep_inputs(**inputs)
    total, _ = run_on_cores(in_maps)
    return total



# revision 7
# speedup vs baseline: 1.1653x; 1.1653x over previous
"""Trainium2 Bass kernel for nn_AE_RNN: 2-layer GRU AE-RNN, scalar SSE loss.

Data-parallel over batch (256 -> 32 per core, 8 cores). Host sums partials.

v2: merged-layer lockstep recurrence. Iteration i computes layer0 step i
("A", cols 0:32) and layer1 step i-1 ("B", cols 32:64) in shared [128,64]
elementwise ops, halving per-step instruction count on the serial chain.
  - gate input biases gi0 (composed W@relu1) preloaded into PSUM via
    identity matmuls; gate matmuls accumulate on top (start=False).
  - z-gate weights negated so sigmoid gives zc = 1-z directly.
  - tail composes out the xcat stage (px_w1 @ WxcP).
  - state stream HS holds packed [h0|h1] per step; tail reads the h1
    halves via a strided 3D rhs AP.
"""

import numpy as np
import ml_dtypes

import concourse.bass as bass
import concourse.bacc as bacc
import concourse.mybir as mybir
import concourse.tile as tile
from concourse.bass_utils import run_bass_kernel_spmd

BF16 = mybir.dt.bfloat16
F32 = mybir.dt.float32
AF = mybir.ActivationFunctionType
OP = mybir.AluOpType

B, T = 256, 2048
U = Y = 32
H = 128
Z = 64
NCORE = 8
BL = B // NCORE  # 32 batch rows per core

C = 128  # steps per chunk
NCH = T // C


def _bf(x):
    return np.asarray(x, dtype=ml_dtypes.bfloat16)


def build_nc(nch=NCH):
    nc = bacc.Bacc("TRN2", target_bir_lowering=False, debug=False)
    Tl = nch * C
    NT = (C * BL) // 512  # 512-wide tiles per chunk = 8

    u_d = nc.dram_tensor("u", [U, Tl, BL], BF16, kind="ExternalInput")
    y_d = nc.dram_tensor("y", [Y, Tl, BL], BF16, kind="ExternalInput")
    h00_d = nc.dram_tensor("h00", [H, BL], F32, kind="ExternalInput")
    h01_d = nc.dram_tensor("h01", [H, BL], F32, kind="ExternalInput")
    wp_d = nc.dram_tensor("wpack", [128, 19 * 128], BF16, kind="ExternalInput")
    bp_d = nc.dram_tensor("bpack", [128, 8], F32, kind="ExternalInput")
    out_d = nc.dram_tensor("out", [1, 1], F32, kind="ExternalOutput")

    with tile.TileContext(nc) as tc:
        with (
            tc.tile_pool(name="wt", bufs=1) as wtp,
            tc.tile_pool(name="ut", bufs=2) as utp,
            tc.tile_pool(name="yt", bufs=3) as ytp,
            tc.tile_pool(name="big", bufs=2) as bigp,
            tc.tile_pool(name="r1", bufs=3) as r1p,
            tc.tile_pool(name="hs", bufs=3) as hsp,
            tc.tile_pool(name="sm", bufs=4) as smp,
            tc.tile_pool(name="tl", bufs=2) as tlp,
            tc.tile_pool(name="prz", bufs=2, space="PSUM") as przp,
            tc.tile_pool(name="pn", bufs=2, space="PSUM") as pnp,
            tc.tile_pool(name="psb", bufs=2, space="PSUM") as psb,
        ):
            wp = wtp.tile([128, 19 * 128], BF16)
            bp = wtp.tile([128, 8], F32)
            nc.sync.dma_start(wp[:], wp_d[:])
            nc.sync.dma_start(bp[:], bp_d[:])

            def W(i, n=128):
                return wp[:, i * 128:i * 128 + n]

            def Bc(j, p=128):
                return bp[:p, j:j + 1]

            I128 = W(18)

            ones32 = wtp.tile([32, 1], F32)
            nc.vector.memset(ones32[:], 1.0)
            onesb = wtp.tile([128, 64], BF16)
            nc.vector.memset(onesb[:], 1.0)

            h0f = wtp.tile([H, BL], F32)
            h1f = wtp.tile([H, BL], F32)
            nc.sync.dma_start(h0f[:], h00_d[:])
            nc.sync.dma_start(h1f[:], h01_d[:])

            loss_acc = wtp.tile([32, NT * nch], F32)
            nc.vector.memset(loss_acc[:], 0.0)

            # ---- bulk pass: relu1 + gi0 (r/z/n) for a chunk ----
            def bulk(k, Ut):
                relu1 = r1p.tile([H, C * BL], BF16, tag="relu1")
                for j in range(NT):
                    ps = psb.tile([128, 512], F32, tag="pb")
                    nc.tensor.matmul(ps[:], W(0)[0:32, :],
                                     Ut[:, j * 512:(j + 1) * 512])
                    nc.scalar.activation(relu1[:, j * 512:(j + 1) * 512], ps[:],
                                         AF.Relu, bias=Bc(0))
                gbufs = []
                for g in range(3):
                    gb = bigp.tile([H, C * BL], BF16, tag=f"g0_{g}")
                    for j in range(NT):
                        ps = psb.tile([128, 512], F32, tag="pb")
                        nc.tensor.matmul(ps[:], W(1 + g),
                                         relu1[:, j * 512:(j + 1) * 512])
                        nc.scalar.activation(gb[:, j * 512:(j + 1) * 512],
                                             ps[:], AF.Identity, bias=Bc(1 + g))
                    gbufs.append(gb)
                return relu1, gbufs

            # ---- tail for one finished chunk ----
            def tail(prev, k):
                relu1, HS_t, Yt = prev
                hsv = HS_t[:].rearrange("p (t x) -> p t x", x=64)
                for j in range(NT):
                    sl = slice(j * 512, (j + 1) * 512)
                    h1rhs = hsv[:, j * 16 + 1:j * 16 + 17, 32:64]
                    ps = psb.tile([128, 512], F32, tag="pb")
                    nc.tensor.matmul(ps[:], W(13), relu1[:, sl],
                                     start=True, stop=False)
                    nc.tensor.matmul(ps[:], W(14), h1rhs,
                                     start=False, stop=True)
                    dphi = tlp.tile([128, 512], BF16, tag="dphi")
                    nc.scalar.activation(dphi[:], ps[:], AF.Relu, bias=Bc(4))
                    ps2 = psb.tile([128, 512], F32, tag="pb")
                    nc.tensor.matmul(ps2[:], W(15), dphi[:])
                    px = tlp.tile([128, 512], BF16, tag="px")
                    nc.scalar.activation(px[:], ps2[:], AF.Relu, bias=Bc(5))
                    ps3 = psb.tile([128, 512], F32, tag="pb")
                    nc.tensor.matmul(ps3[:], W(16), px[:])
                    me = tlp.tile([128, 512], BF16, tag="me")
                    nc.scalar.activation(me[:], ps3[:], AF.Relu, bias=Bc(6))
                    ps4t = psb.tile([128, 512], F32, tag="pb")
                    ps4 = ps4t[0:32, :]
                    nc.tensor.matmul(ps4, W(17, 32), me[:])
                    d = tlp.tile([32, 512], F32, tag="dd")
                    nc.vector.scalar_tensor_tensor(d[:], ps4, Bc(7, 32),
                                                   Yt[:, sl], op0=OP.add,
                                                   op1=OP.subtract)
                    d2 = tlp.tile([32, 512], F32, tag="d2")
                    nc.scalar.activation(
                        d2[:], d[:], AF.Square,
                        accum_out=loss_acc[:, k * NT + j:k * NT + j + 1])

            # ---- prologue: DMA chunk 0 inputs, bulk for chunk 0 ----
            Ut = utp.tile([U, C * BL], BF16, tag="ut")
            Yt = ytp.tile([Y, C * BL], BF16, tag="yt")
            nc.sync.dma_start(Ut[:].rearrange("p (t b) -> p t b", b=BL),
                              u_d[:, 0:C, :])
            nc.sync.dma_start(Yt[:].rearrange("p (t b) -> p t b", b=BL),
                              y_d[:, 0:C, :])
            relu1, (G0R, G0Z, G0N) = bulk(0, Ut)

            HS_prev_tile = None
            prev = None  # (relu1, HS, Yt) of previous chunk for tail

            for k in range(nch):
                HS = hsp.tile([H, (C + 1) * 64], BF16, tag="hs")
                if k == 0:
                    # seed slot1 halves written by special iter 0 + init copies
                    nc.vector.tensor_copy(HS[:, 0:32], h0f[:])
                    nc.vector.tensor_copy(HS[:, 96:128], h1f[:])

                # next chunk: DMA + bulk (scheduler interleaves with steps)
                if k + 1 < nch:
                    Ut_n = utp.tile([U, C * BL], BF16, tag="ut")
                    Yt_n = ytp.tile([Y, C * BL], BF16, tag="yt")
                    nc.sync.dma_start(
                        Ut_n[:].rearrange("p (t b) -> p t b", b=BL),
                        u_d[:, (k + 1) * C:(k + 2) * C, :])
                    nc.sync.dma_start(
                        Yt_n[:].rearrange("p (t b) -> p t b", b=BL),
                        y_d[:, (k + 1) * C:(k + 2) * C, :])
                    nxt = bulk(k + 1, Ut_n)
                else:
                    Ut_n = Yt_n = None
                    nxt = None

                for s in range(C):
                    i = k * C + s  # global iteration
                    first = (i == 0)
                    if s == 0:
                        hprev = (HS[:, 0:64] if k == 0 else
                                 HS_prev_tile[:, C * 64:(C + 1) * 64])
                    else:
                        hprev = HS[:, s * 64:(s + 1) * 64]
                    h0p = hprev[:, 0:32]
                    h1p = hprev[:, 32:64]
                    s32 = slice(s * 32, (s + 1) * 32)

                    # --- PSUM rz: [rA rB zA zB] ---
                    rz = przp.tile([128, 128], F32, tag="rz")
                    nc.tensor.matmul(rz[:, 0:32], I128, G0R[:, s32],
                                     start=True, stop=False)
                    nc.tensor.matmul(rz[:, 0:32], W(4), h0p,
                                     start=False, stop=True)
                    if not first:
                        nc.tensor.matmul(rz[:, 32:64], W(7), h0p,
                                         start=True, stop=False)
                        nc.tensor.matmul(rz[:, 32:64], W(10), h1p,
                                         start=False, stop=True)
                    nc.tensor.matmul(rz[:, 64:96], I128, G0Z[:, s32],
                                     start=True, stop=False)
                    nc.tensor.matmul(rz[:, 64:96], W(5), h0p,
                                     start=False, stop=True)
                    if not first:
                        nc.tensor.matmul(rz[:, 96:128], W(8), h0p,
                                         start=True, stop=False)
                        nc.tensor.matmul(rz[:, 96:128], W(11), h1p,
                                         start=False, stop=True)

                    # --- PSUM n: [ghnA ghnB ginA ginB] ---
                    nps = pnp.tile([128, 128], F32, tag="n")
                    nc.tensor.matmul(nps[:, 0:32], W(6), h0p)
                    nc.tensor.matmul(nps[:, 64:96], I128, G0N[:, s32])
                    if not first:
                        nc.tensor.matmul(nps[:, 32:64], W(12), h1p)
                        nc.tensor.matmul(nps[:, 96:128], W(9), h0p)

                    wd = 32 if first else 64
                    # --- sigmoids ---
                    rs = smp.tile([128, 64], BF16, tag="rs")
                    nc.scalar.activation(rs[:, 0:wd], rz[:, 0:wd], AF.Sigmoid)
                    zs = smp.tile([128, 64], BF16, tag="zs")
                    nc.scalar.activation(zs[:, 0:wd], rz[:, 64:64 + wd],
                                         AF.Sigmoid)
                    # --- n path ---
                    tn = smp.tile([128, 64], BF16, tag="tn")
                    nc.vector.tensor_tensor(tn[:, 0:wd], rs[:, 0:wd],
                                            nps[:, 0:wd], op=OP.mult)
                    tnb = smp.tile([128, 64], BF16, tag="tnb")
                    nc.vector.tensor_tensor(tnb[:, 0:wd], tn[:, 0:wd],
                                            nps[:, 64:64 + wd], op=OP.add)
                    ns = smp.tile([128, 64], BF16, tag="ns")
                    nc.scalar.activation(ns[:, 0:wd], tnb[:, 0:wd], AF.Tanh)
                    # --- blend: h = zc*n + (1-zc)*h_prev ---
                    zi = smp.tile([128, 64], BF16, tag="zi")
                    nc.gpsimd.tensor_tensor(zi[:, 0:wd], onesb[:, 0:wd],
                                            zs[:, 0:wd], op=OP.subtract)
                    bb = smp.tile([128, 64], BF16, tag="bb")
                    nc.gpsimd.tensor_tensor(bb[:, 0:wd], zi[:, 0:wd],
                                            hprev[:, 0:wd], op=OP.mult)
                    aa = smp.tile([128, 64], BF16, tag="aa")
                    nc.vector.tensor_tensor(aa[:, 0:wd], zs[:, 0:wd],
                                            ns[:, 0:wd], op=OP.mult)
                    dst = HS[:, (s + 1) * 64:(s + 1) * 64 + wd]
                    nc.vector.tensor_tensor(dst, aa[:, 0:wd], bb[:, 0:wd],
                                            op=OP.add)

                if prev is not None:
                    tail(prev, k - 1)
                prev = (relu1, HS, Yt)
                HS_prev_tile = HS
                if nxt is not None:
                    relu1, (G0R, G0Z, G0N) = nxt
                    Ut, Yt = Ut_n, Yt_n

            tail(prev, nch - 1)

            # ---- final loss reduction ----
            lsum = smp.tile([32, 1], F32, tag="lsum")
            nc.vector.tensor_reduce(lsum[:], loss_acc[:],
                                    axis=mybir.AxisListType.X, op=OP.add)
            pslt = psb.tile([128, 512], F32, tag="pb")
            psl = pslt[0:1, 0:1]
            nc.tensor.matmul(psl, ones32[:], lsum[:])
            lout = smp.tile([1, 1], F32, tag="lout")
            nc.vector.tensor_copy(lout[:], psl)
            nc.sync.dma_start(out_d[:], lout[:])

    nc.compile()
    return nc


def prep_inputs(u, y, h0, pu_w1, pu_b1, pu_w2, pu_b2, dy_w1, dy_b1, dy_w2,
                dy_b2, xm_w, xm_b, xlv_w, xlv_b, px_w1, px_b1, px_w2, px_b2,
                me_w1, me_b1, me_w2, me_b2, gru_wih, gru_whh, t_steps=T):
    """Host-side: compose weights, build wpack/bpack, per-core input shards."""
    f = np.float32
    asf = lambda x: np.array(x, f)
    (u, y, h0, pu_w1, pu_b1, pu_w2, pu_b2, dy_w1, dy_b1, dy_w2, dy_b2, xm_w,
     xm_b, xlv_w, xlv_b, px_w1, px_b1, px_w2, px_b2, me_w1, me_b1, me_w2,
     me_b2, gru_wih, gru_whh) = map(asf, (
         u, y, h0, pu_w1, pu_b1, pu_w2, pu_b2, dy_w1, dy_b1, dy_w2, dy_b2,
         xm_w, xm_b, xlv_w, xlv_b, px_w1, px_b1, px_w2, px_b2, me_w1, me_b1,
         me_w2, me_b2, gru_wih, gru_whh))
    wih0, wih1 = gru_wih[0].copy(), gru_wih[1].copy()
    whh0, whh1 = gru_whh[0].copy(), gru_whh[1].copy()
    # negate z-gate (rows H:2H) so sigmoid gives zc = 1-z
    for w in (wih0, whh0, wih1, whh1):
        w[H:2 * H] *= -1.0

    Wg0 = wih0 @ pu_w2
    bg0 = wih0 @ pu_b2
    Wda = dy_w1[:, :H] @ pu_w2
    bda = dy_w1[:, :H] @ pu_b2 + dy_b1
    dyw1b = dy_w1[:, H:]
    Wxc = np.vstack([xm_w, xlv_w])
    WxcP = Wxc @ dy_w2
    bxcP = Wxc @ dy_b2 + np.concatenate([xm_b, xlv_b])
    # compose xcat away: px = relu(Wpx @ dphi + bpx)
    Wpx = px_w1 @ WxcP
    bpx = px_w1 @ bxcP + px_b1
    WmeP = me_w1 @ px_w2
    bmeP = me_w1 @ px_b2 + me_b1

    wpack = np.zeros((128, 19 * 128), f)

    def put(i, w):  # w: [out, in] -> lhsT [in, out]
        wt = np.ascontiguousarray(w.T)
        wpack[:wt.shape[0], i * 128:i * 128 + wt.shape[1]] = wt

    put(0, pu_w1)
    for g in range(3):
        put(1 + g, Wg0[g * H:(g + 1) * H])
        put(4 + g, whh0[g * H:(g + 1) * H])
        put(7 + g, wih1[g * H:(g + 1) * H])
        put(10 + g, whh1[g * H:(g + 1) * H])
    put(13, Wda)
    put(14, dyw1b)
    put(15, Wpx)
    put(16, WmeP)
    put(17, me_w2)
    put(18, np.eye(128, dtype=f))

    bpack = np.zeros((128, 8), f)
    for j, b in enumerate([pu_b1, bg0[:H], bg0[H:2 * H], bg0[2 * H:],
                           bda, bpx, bmeP, me_b2]):
        bpack[:len(b), j] = b

    in_maps = []
    for c in range(NCORE):
        rows = slice(c * BL, (c + 1) * BL)
        in_maps.append({
            "u": _bf(u[rows, :, :t_steps].transpose(1, 2, 0)),  # [U, T, BL]
            "y": _bf(y[rows, :, :t_steps].transpose(1, 2, 0)),
            "h00": np.ascontiguousarray(h0[0, rows].T),
            "h01": np.ascontiguousarray(h0[1, rows].T),
            "wpack": _bf(wpack),
            "bpack": bpack,
        })
    return in_maps


_NC_CACHE = {}


def run_on_cores(in_maps, nch=NCH, trace=False, tmpdir=None):
    key = nch
    if key not in _NC_CACHE:
        _NC_CACHE[key] = build_nc(nch)
    nc = _NC_CACHE[key]
    res = run_bass_kernel_spmd(nc, in_maps, core_ids=list(range(NCORE)),
                               trace=trace, tmpdir=tmpdir)
    total = np.float32(0.0)
    for r in res.results:
        total += np.float32(r["out"][0, 0])
    return np.float32(total), res


def kernel(**inputs):
    in_maps = prep_inputs(**inputs)
    total, _ = run_on_cores(in_maps)
    return total
